# revision 57
# baseline (speedup 1.0000x reference)
"""Causal self-attention on 8 Trainium2 NeuronCores.

Problem: x[2, 2048, 1024], 16 heads (head_size 64),
  qkv = x @ w_attn + b_attn; causal softmax attention; y @ w_proj + b_proj.

Sharding: 8 cores = 2 (batch) x 4 (head groups of 4 heads).  Core c handles
batch b = c // 4 and heads [4*hg, 4*hg + 4) with hg = c % 4.  The projection
is row-parallel (each core contracts its 256 y-columns against its w_proj row
slice), so each core emits a partial [1024, 2048] outT; the host sums the 4
partials per batch, transposes, and adds b_proj.

Layout trick: the host feeds x[b].T (bf16) so every on-chip matmul consumes
natural layouts (contraction on partitions) and no PE transposes are needed:
  qkT[c', t]  = wqk.T @ xT           (lhsT = wqk [C, 512],  rhs = xT)
  v'[t, d']   = xT.T  @ wv           (lhsT = xT, rhs = wv; 65 cols per head,
                                      column 64 preset to 1.0)
  sT[j, i]    = kT_h.T @ qT_h        (K = 64, two heads row-packed into the
                                      PE array; one 2-bank PSUM tile per pair)
  attT        = exp(sT / 8)          (one ScalarE op per pair; causal mask via
                                      gpsimd affine_select on diagonal blocks)
  yT'[d', i]  = v'_h.T @ attT_h      (M = 65: row 64 accumulates the softmax
                                      denominator for free)
  yT          = yT'[0:64] / yT'[64]  (DVE recip -> partition_broadcast -> mult)
  outT[c, t]  = wp.T @ yT            (lhsT = wp [256, 1024], rhs = yT)
All matmuls are bf16 with fp32 PSUM accumulation; causality skips 24 of 64
S^T/AV block-columns; the attention inner loop is software-pipelined so PE
issues S^T(jt+1) while ScalarE computes exp(jt).
"""

import ml_dtypes
import numpy as np

P = 128
B, T, C = 2, 2048, 1024
N_HEAD = 16
HSZ = C // N_HEAD          # 64
HG = 4                     # heads per core
DQK = 2 * HG * HSZ         # 512 (q cols + k cols per core)
DV = HG * HSZ              # 256 (v cols per core)
KSUB = C // P              # 8  k-subtiles for the C contraction
ICH = 512                  # i-chunk (PSUM free dim)
NIC = T // ICH             # 4
NJT = T // P               # 16 j-tiles
SCALE = 1.0 / np.sqrt(HSZ)  # 0.125

_CACHE = {}


def _build(reps=1, loop_reps=1, body="full"):
    """body: 'full' (the real kernel), or diagnostics: 'noio' (input DMAs
    hoisted out of the timing loop, no output DMA), 'empty' (tiny DMA
    round-trip only — measures For_i loop overhead)."""
    import concourse.bacc as bacc
    import concourse.mybir as mybir
    import concourse.tile as tile

    f32 = mybir.dt.float32
    bf16 = mybir.dt.bfloat16
    f32r = mybir.dt.float32r
    AF = mybir.ActivationFunctionType
    ALU = mybir.AluOpType

    nc = bacc.Bacc("TRN2", debug=False, target_bir_lowering=False)

    xT_d = nc.dram_tensor("xT", [C, T], bf16, kind="ExternalInput").ap()
    # m-major host layout: wqk[m, p, ko*128+c] = W[ko*128+p, m*128+c] so each
    # 128-col m-block loads as ONE DMA with 2KB-contiguous segments per
    # partition on both sides
    wqk_d = nc.dram_tensor("wqk", [DQK // P, P, C], bf16,
                           kind="ExternalInput").ap()
    wv_d = nc.dram_tensor("wv", [C, DV], bf16, kind="ExternalInput").ap()
    wp_d = nc.dram_tensor("wp", [DV, C], bf16, kind="ExternalInput").ap()
    bqk_d = nc.dram_tensor("bqk", [DQK], f32, kind="ExternalInput").ap()
    bv_d = nc.dram_tensor("bv", [DV], f32, kind="ExternalInput").ap()
    out_d = nc.dram_tensor("outT", [C, T], bf16, kind="ExternalOutput").ap()

    if body == "empty":
        with tile.TileContext(nc) as tc:
            with tc.tile_pool(name="p", bufs=2) as pool:
                _hints = (mybir.EngineType.PE, mybir.EngineType.DVE,
                          mybir.EngineType.Activation, mybir.EngineType.Pool,
                          mybir.EngineType.SP)
                from contextlib import nullcontext
                loop_ctx = (tc.For_i(0, loop_reps, 1, hint_engines=_hints)
                            if loop_reps > 1 else nullcontext())
                with loop_ctx:
                    t = pool.tile([1, 64], bf16, name="t")
                    nc.sync.dma_start(t[:], xT_d[0:1, 0:64])
                    nc.sync.dma_start(out_d[0:1, 0:64], t[:])
        nc.compile()
        return nc

    if body in ("dma1", "dma2"):
        # DMA bandwidth probes: per-iteration transfer of the full 4MB xT
        # through one hwdge queue (dma1) or split across SP+Activation (dma2)
        with tile.TileContext(nc) as tc:
            with tc.tile_pool(name="p", bufs=1) as pool:
                _hints = (mybir.EngineType.PE, mybir.EngineType.DVE,
                          mybir.EngineType.Activation, mybir.EngineType.Pool,
                          mybir.EngineType.SP)
                from contextlib import nullcontext
                loop_ctx = (tc.For_i(0, loop_reps, 1, hint_engines=_hints)
                            if loop_reps > 1 else nullcontext())
                xt = pool.tile([P, KSUB, T], bf16, name="xt")
                xT_r = xT_d.rearrange("(ko p) t -> p ko t", p=P)
                with loop_ctx:
                    for k in range(0, KSUB, 2):
                        eng = (nc.sync if (body == "dma1" or (k // 2) % 2 == 0)
                               else nc.scalar)
                        eng.dma_start(xt[:, k:k + 2, :], xT_r[:, k:k + 2, :])
                    nc.sync.dma_start(out_d[0:1, 0:64], xt[0:1, 0, 0:64])
        nc.compile()
        return nc

    with tile.TileContext(nc) as tc:
        with (
            tc.tile_pool(name="consts", bufs=1) as consts,
            tc.tile_pool(name="attp", bufs=10) as attp,
            tc.tile_pool(name="recp", bufs=6) as recp,
            tc.tile_pool(name="obp", bufs=2) as obp,
            tc.tile_pool(name="bcp", bufs=4) as bcp,
            tc.tile_pool(name="st_ps", bufs=2, space="PSUM") as st_ps,
            tc.tile_pool(name="yt_ps", bufs=2, space="PSUM") as yt_ps,
            tc.tile_pool(name="pj_ps", bufs=2, space="PSUM") as pj_ps,
        ):
          from contextlib import nullcontext
          _hints = (mybir.EngineType.PE, mybir.EngineType.DVE,
                    mybir.EngineType.Activation, mybir.EngineType.Pool,
                    mybir.EngineType.SP)
          loop_ctx = (tc.For_i(0, loop_reps, 1, hint_engines=_hints)
                      if loop_reps > 1 else nullcontext())

          def load_inputs():
            # ---------------- input DMA (ordered by first use) ----------------
            xt = consts.tile([P, KSUB, T], bf16, name="xt")
            xT_r = xT_d.rearrange("(ko p) t -> p ko t", p=P)
            wqk = consts.tile([P, DQK // P, KSUB, P], bf16, name="wqk")
            # load order = first use: the first attention call gates on
            # m0 (q01) + m2 (k01) + xt cc0 + wv/bqk only (~2.5 MB); m1/m3
            # backfill right after.  Every DMA moves >=1KB-contiguous
            # segments per partition (sub-KB scatter throttles the queue).
            bqk = consts.tile([P, DQK // P], f32, name="bqk")
            wv = consts.tile([P, KSUB, DV], bf16, name="wv")
            bv_row = consts.tile([1, DV], f32, name="bv_row")

            def wqk_m(m, half=None):
                ksl = (slice(None) if half is None
                       else slice(half * (KSUB // 2), (half + 1) * (KSUB // 2)))
                wr = wqk_d[m].rearrange("p (ko c) -> p ko c", c=P)
                nc.sync.dma_start(wqk[:, m, ksl], wr[:, ksl])
            nc.sync.dma_start(xt[:, 0:2, 0:ICH], xT_r[:, 0:2, 0:ICH])
            wqk_m(0, 0)
            nc.sync.dma_start(bqk[:], bqk_d.rearrange("(m p) -> p m", p=P))
            nc.sync.dma_start(xt[:, 2:4, 0:ICH], xT_r[:, 2:4, 0:ICH])
            wqk_m(0, 1)
            nc.sync.dma_start(wv[:], wv_d.rearrange("(ko p) n -> p ko n", p=P))
            nc.sync.dma_start(xt[:, 4:6, 0:ICH], xT_r[:, 4:6, 0:ICH])
            wqk_m(2, 0)
            nc.sync.dma_start(xt[:, 6:8, 0:ICH], xT_r[:, 6:8, 0:ICH])
            wqk_m(2, 1)
            wqk_m(1)
            wqk_m(3)
            nc.sync.dma_start(bv_row[:], bv_d[None, :])
            for cc in range(1, NIC):
                nc.sync.dma_start(
                    xt[:, :, cc * ICH:(cc + 1) * ICH],
                    xT_r[:, :, cc * ICH:(cc + 1) * ICH],
                )
            wp = consts.tile([P, DV // P, C], bf16, name="wp")
            nc.sync.dma_start(wp[:], wp_d.rearrange("(ko p) m -> p ko m", p=P))
            bv_bc = consts.tile([P, DV], f32, name="bv_bc")
            nc.gpsimd.partition_broadcast(bv_bc[:], bv_row[:])
            return xt, wqk, bqk, wv, wp, bv_bc

          if body == "noio":
            xt, wqk, bqk, wv, wp, bv_bc = load_inputs()

          # ---- loop-invariant setup (outside For_i: built exactly once) ----
          # preload the Exp activation table so the first real exp doesn't
          # pay the 1.3us table load
          warm = consts.tile([1, 2], f32, name="warm")
          nc.gpsimd.memset(warm[:], 0.0)
          nc.scalar.activation(warm[0:1, 0:1], warm[0:1, 1:2],
                               AF.Exp, scale=SCALE)
          # persistent activations
          qk = consts.tile([P, 4, T], bf16, name="qk")     # m: q01 q23 k01 k23
          v = consts.tile([P, NJT, 4 * (HSZ + 1)], bf16, name="v")
          # only the c=64 column of each head block must be 1.0 (softmax
          # denominator rides the extra matmul row); the rest is written by
          # the bias add
          nc.gpsimd.memset(
              v[:].rearrange("p j (h c) -> p j h c", c=HSZ + 1)[:, :, :, HSZ:],
              1.0)
          yt = consts.tile([P, 2, T], bf16, name="yt")

          with loop_ctx:
           for _rep in range(reps):
            if body in ("full", "serial"):
                xt, wqk, bqk, wv, wp, bv_bc = load_inputs()
            if body == "serial":
                # cross-iteration serializer (bench diagnostic only): gate the
                # first compute of this iteration on the previous iteration's
                # final output DMA, so the For_i slope measures serial latency
                # rather than pipelined throughput.
                dummy = recp.tile([1, 64], bf16, tag="rec", name="dummy")
                nc.sync.dma_start(dummy[:], out_d[None, 0, T - 64:T])
                zb = recp.tile([1, 1], bf16, tag="rec", name="zb")
                nc.vector.tensor_scalar_mul(zb[:], dummy[0:1, 0:1], 0.0)
                nc.vector.tensor_tensor(wqk[0:1, 0, 0, 0:1],
                                        wqk[0:1, 0, 0, 0:1], zb[:], ALU.add)

            # ---- phase emitters (order below controls PE stream / overlap) ----
            def emit_qkT(m, cc):
                # shares the pj pool (1-bank tiles) so fillers never steal the
                # attention pipeline's st_ps buffers
                ps = pj_ps.tile([P, ICH], f32, tag="pj", name="qk_ps")
                for k in range(KSUB):
                    nc.tensor.matmul(
                        ps[:],
                        lhsT=wqk[:, m, k, :],
                        rhs=xt[:, k, cc * ICH:(cc + 1) * ICH],
                        start=(k == 0),
                        stop=(k == KSUB - 1),
                    )
                nc.vector.tensor_tensor(
                    qk[:, m, cc * ICH:(cc + 1) * ICH], ps[:],
                    bqk[:, m:m + 1].to_broadcast([P, ICH]), ALU.add,
                )

            def emit_v(t):
                ps = pj_ps.tile([P, ICH], f32, tag="pj", name="v_ps")
                for k in range(KSUB):
                    nc.tensor.matmul(
                        ps[0:P, 0:DV],
                        lhsT=xt[:, k, t * P:(t + 1) * P],
                        rhs=wv[:, k, :],
                        start=(k == 0),
                        stop=(k == KSUB - 1),
                    )
                nc.vector.tensor_tensor(
                    v[:, t, :].rearrange("p (h c) -> p h c", c=HSZ + 1)[:, :, 0:HSZ],
                    ps[0:P, 0:DV].rearrange("p (h c) -> p h c", c=HSZ),
                    bv_bc[:].rearrange("p (h c) -> p h c", c=HSZ),
                    ALU.add,
                )

            def emit_attn(ic, hps, extra=(), direct_norm=False):
                """Attention for i-chunk ic over head-pairs in hps.  `extra`
                is a list of thunks (projection units of the previous i-chunk)
                emitted one per j-tile step so the PE has filler work while
                ScalarE computes the exp."""
                isl = slice(ic * ICH, (ic + 1) * ICH)
                njt = 4 * ic + 4          # causal: j-tiles 0 .. 4*ic+3
                HB = HSZ + 1
                extra = list(extra)
                ytp = {hp: (yt_ps.tile([P, ICH], f32, tag="yt", name="ytpA"),
                            yt_ps.tile([P, ICH], f32, tag="yt", name="ytpB"))
                       for hp in hps}

                def emit_st(hp, jt):
                    jsl = slice(jt * P, (jt + 1) * P)
                    # diagonal blocks: columns f < 128r are masked for every
                    # partition, so compute only the valid suffix [n0:ICH)
                    r = jt - 4 * ic if jt >= 4 * ic else None
                    n0 = 0 if r is None else P * r
                    ssl = slice(ic * ICH + n0, (ic + 1) * ICH)
                    st2 = st_ps.tile([P, 2, ICH], f32, tag="st", name="st2")
                    for h in range(2):
                        hsl = slice(64 * h, 64 * h + 64)
                        nc.tensor.matmul(
                            st2[:, h, n0:],
                            lhsT=qk[hsl, 2 + hp, jsl],
                            rhs=qk[hsl, hp, ssl],
                        )
                    a2 = attp.tile([P, 2, ICH], bf16, tag="att", name="a2")
                    nc.scalar.activation(a2[:, :, n0:], st2[:, :, n0:],
                                         AF.Exp, scale=SCALE)
                    if r is not None:
                        # only the 128-wide band [n0, n0+128) is partial; the
                        # rest of the suffix is fully valid.  within the band
                        # keep f >= p.  Runs on Pool, off the PE pipeline; the
                        # depth-2 AV delay hides its latency.
                        mb = slice(n0, min(n0 + P, ICH))
                        nc.gpsimd.affine_select(
                            out=a2[:, :, mb], in_=a2[:, :, mb],
                            compare_op=ALU.is_ge, fill=0.0,
                            base=0, channel_multiplier=-1,
                            pattern=[[0, 2], [1, mb.stop - mb.start]])
                    return a2, n0

                def emit_av(hp, jt, a2, n0):
                    first, last = jt == 0, jt == njt - 1
                    ytpA, ytpB = ytp[hp]
                    nc.tensor.matmul(
                        ytpA[0:HB, n0:],
                        lhsT=v[:, jt, (2 * hp) * HB:(2 * hp + 1) * HB],
                        rhs=a2[:, 0, n0:],
                        start=first, stop=last,
                    )
                    nc.tensor.matmul(
                        ytpB[0:HB, n0:],
                        lhsT=v[:, jt, (2 * hp + 1) * HB:(2 * hp + 2) * HB],
                        rhs=a2[:, 1, n0:],
                        start=first, stop=last,
                    )

                # software pipeline depth 2: AV(jt) issues two steps after its
                # S^T, so the Act->PE semaphore latency is absorbed by the
                # queue instead of stalling the PE
                pend = []
                for jt in range(njt):
                    cur = [(hp,) + tuple(emit_st(hp, jt)) for hp in hps]
                    if len(pend) >= 2:
                        pjt, pcur = pend.pop(0)
                        for hp, a2, n0 in pcur:
                            emit_av(hp, pjt, a2, n0)
                    if extra:
                        extra.pop(0)()
                    pend.append((jt, cur))
                for pjt, pcur in pend:
                    for hp, a2, n0 in pcur:
                        emit_av(hp, pjt, a2, n0)
                for th in extra:
                    th()

                for hp in hps:
                    ytpA, ytpB = ytp[hp]
                    if direct_norm:
                        # final attn call: no successor competes for these
                        # psum banks, so normalize straight out of PSUM —
                        # shortens the chain into the final projection
                        ycA, ycB = ytpA, ytpB
                    else:
                        # fast-evict to SBUF: frees the psum accumulators;
                        # keeps ScalarE free for the exp stream
                        ycA = recp.tile([P, ICH], f32, tag="rec", name="ycA")
                        ycB = recp.tile([P, ICH], f32, tag="rec", name="ycB")
                        nc.vector.tensor_copy(ycA[0:HB, :], ytpA[0:HB, :])
                        nc.vector.tensor_copy(ycB[0:HB, :], ytpB[0:HB, :])
                    # shifted DVE write puts the recip at partition 0 of a
                    # fresh tile, where partition_broadcast needs its source
                    recA = bcp.tile([P, ICH], f32, tag="rec2", name="recA")
                    recB = bcp.tile([P, ICH], f32, tag="rec2", name="recB")
                    nc.vector.reciprocal(recA[0:1, :], ycA[64:65, :])
                    nc.vector.reciprocal(recB[0:1, :], ycB[64:65, :])
                    # only partitions 0:64 are read by the mults below
                    rbA = bcp.tile([P, ICH], f32, tag="rb", name="rbA")
                    rbB = bcp.tile([P, ICH], f32, tag="rb", name="rbB")
                    nc.gpsimd.partition_broadcast(rbA[0:64, :], recA[0:1, :])
                    nc.gpsimd.partition_broadcast(rbB[0:64, :], recB[0:1, :])
                    nc.vector.tensor_tensor(yt[0:64, hp, isl], ycA[0:64, :],
                                            rbA[0:64, :], ALU.mult)
                    nc.vector.tensor_tensor(yt[64:128, hp, isl], ycB[0:64, :],
                                            rbB[0:64, :], ALU.mult)

            def proj_units(ic, stream_dma=False, split_k_first=0):
                """split_k_first: that many leading m-units are emitted as
                (k2=0 half now, k2=1 half at the next unit's slot) so their
                first halves run while the hp1 normalize chain drains —
                their psum tiles stay live across the gap (needs <= pj bufs).
                """
                isl = slice(ic * ICH, (ic + 1) * ICH)
                ob = obp.tile([P, C // P, ICH], bf16, tag="ob", name="ob")
                out_r = out_d.rearrange("(m p) t -> p m t", p=P)
                held = {}

                def mm(pj, m, k2):
                    nc.tensor.matmul(
                        pj[:],
                        lhsT=wp[:, k2, m * P:(m + 1) * P],
                        rhs=yt[:, k2, isl],
                        start=(k2 == 0),
                        stop=(k2 == DV // P - 1),
                    )

                def finish(m, pj):
                    nc.vector.tensor_copy(ob[:, m, :], pj[:])
                    if body in ("full", "serial"):
                        if stream_dma and (m + 1) % stream_dma == 0:
                            # late chunks: stream out in m-groups so the
                            # final DMA is small, not 2 MB of tail
                            lo = m + 1 - stream_dma
                            nc.sync.dma_start(
                                out_r[:, lo:m + 1, isl], ob[:, lo:m + 1, :])
                        elif not stream_dma and m == C // P - 1:
                            # one batched DMA per i-chunk: 8 m-tiles
                            nc.sync.dma_start(out_r[:, :, isl], ob[:])

                def unit(m):
                    def th():
                        if m < split_k_first:
                            # first half only (k2=0, hp0 — no normalize dep)
                            pj = pj_ps.tile([P, ICH], f32, tag="pj", name="pj")
                            mm(pj, m, 0)
                            held[m] = pj
                            return
                        if m - split_k_first in held:
                            pjh = held.pop(m - split_k_first)
                            mm(pjh, m - split_k_first, 1)
                            finish(m - split_k_first, pjh)
                        pj = pj_ps.tile([P, ICH], f32, tag="pj", name="pj")
                        for k2 in range(DV // P):
                            mm(pj, m, k2)
                        finish(m, pj)
                        if m == C // P - 1:
                            for mh in sorted(held):
                                pjh = held.pop(mh)
                                mm(pjh, mh, 1)
                                finish(mh, pjh)
                    return th
                return [unit(m) for m in range(C // P)]

            # ---- emission order: start attention (ScalarE exp) early, and
            # spread every PE-only unit (qkT cc>=1, v jt>=4, projection) as
            # per-jt filler inside the attention loops so the PE never idles
            # while ScalarE drains the exp backlog ----
            def qkT_unit(m, cc):
                return lambda: emit_qkT(m, cc)

            def v_unit(t):
                return lambda: emit_v(t)

            nop = lambda: None  # noqa: E731  boundary slot: let normalize land

            # projections are delayed one full i-chunk (proj(ic) runs as
            # filler during attn(ic+2)) so they never sit behind a freshly
            # issued normalize chain
            emit_qkT(0, 0)
            emit_qkT(2, 0)
            # v(0) rides as the first filler: the attention gate then needs
            # only m0 + m2 + xt cc0 (~2 MB); wv streams in behind
            emit_attn(0, [0], extra=[v_unit(0), v_unit(1), v_unit(2),
                                     v_unit(3), qkT_unit(1, 0)])
            emit_qkT(3, 0)
            emit_attn(0, [1], extra=[qkT_unit(0, 1), qkT_unit(2, 1),
                                     v_unit(4), v_unit(5)])
            emit_attn(1, [0], extra=[qkT_unit(1, 1), qkT_unit(3, 1),
                                     v_unit(6), v_unit(7),
                                     qkT_unit(0, 2), qkT_unit(2, 2)])
            emit_attn(1, [1], extra=[qkT_unit(1, 2), qkT_unit(3, 2),
                                     v_unit(8), v_unit(9)])
            pu0 = proj_units(0)
            emit_attn(2, [0], extra=[qkT_unit(0, 3), qkT_unit(2, 3),
                                     v_unit(10), v_unit(11)] + pu0[:4])
            emit_attn(2, [1], extra=[qkT_unit(1, 3), qkT_unit(3, 3),
                                     v_unit(12), v_unit(13)] + pu0[4:])
            pu1 = proj_units(1)
            emit_attn(3, [0], extra=[nop, v_unit(14), v_unit(15)] + pu1[:6])
            pu2 = proj_units(2, stream_dma=2)
            emit_attn(3, [1], extra=pu1[6:] + pu2[:4], direct_norm=True)
            # pu2 leftovers keep the PE fed while the final normalize chain
            # (DVE recip -> Pool broadcast -> DVE mult) drains
            for th in pu2[4:]:
                th()
            for th in proj_units(NIC - 1, stream_dma=1):
                th()

    nc.compile()
    return nc


def _get_nc(reps=1, loop_reps=1, body="full"):
    key = ("nc", reps, loop_reps, body)
    if key not in _CACHE:
        _CACHE[key] = _build(reps, loop_reps, body)
    return _CACHE[key]


def _shard_inputs(x, w_attn, b_attn, w_proj, b_proj):
    x = np.asarray(x, dtype=np.float32)
    w_attn = np.asarray(w_attn, dtype=np.float32)
    b_attn = np.asarray(b_attn, dtype=np.float32)
    w_proj = np.asarray(w_proj, dtype=np.float32)
    b_proj = np.asarray(b_proj, dtype=np.float32)

    xTs = [np.ascontiguousarray(x[b].T.astype(ml_dtypes.bfloat16)) for b in range(B)]
    in_maps = []
    for core in range(8):
        b, hg = divmod(core, 4)
        q = slice(hg * DV, (hg + 1) * DV)
        k = slice(C + hg * DV, C + (hg + 1) * DV)
        vs = slice(2 * C + hg * DV, 2 * C + (hg + 1) * DV)
        wqk_cm = np.concatenate([w_attn[:, q], w_attn[:, k]], axis=1)  # [C,512]
        # m-major device layout: [m, p, ko*128+c] = wqk_cm[ko*128+p, m*128+c]
        wqk_m = (wqk_cm.reshape(KSUB, P, 4, P).transpose(2, 1, 0, 3)
                 .reshape(4, P, C))
        in_maps.append({
            "xT": xTs[b],
            "wqk": np.ascontiguousarray(wqk_m.astype(ml_dtypes.bfloat16)),
            "wv": np.ascontiguousarray(w_attn[:, vs].astype(ml_dtypes.bfloat16)),
            "wp": np.ascontiguousarray(
                w_proj[hg * DV:(hg + 1) * DV, :].astype(ml_dtypes.bfloat16)),
            "bqk": np.ascontiguousarray(
                np.concatenate([b_attn[q], b_attn[k]])),
            "bv": np.ascontiguousarray(b_attn[vs]),
        })
    return in_maps, b_proj


def _unshard(results, b_proj):
    out = np.zeros((B, T, C), dtype=np.float32)
    for core in range(8):
        b = core // 4
        out[b] += results[core]["outT"].T.astype(np.float32)
    out += b_proj[None, None, :]
    return out


def _run(inputs, **kwargs):
    from concourse.bass_utils import run_bass_kernel_spmd

    nc = _get_nc()
    in_maps, b_proj = _shard_inputs(**inputs)
    res = run_bass_kernel_spmd(nc, in_maps, core_ids=list(range(8)), **kwargs)
    return res, _unshard(res.results, b_proj)


def kernel(x, w_attn, b_attn, w_proj, b_proj):
    _, out = _run(dict(x=x, w_attn=w_attn, b_attn=b_attn,
                       w_proj=w_proj, b_proj=b_proj))
    return out



# revision 61
# speedup vs baseline: 1.2352x; 1.2352x over previous
"""Causal self-attention on 8 Trainium2 NeuronCores.

Problem: x[2, 2048, 1024], 16 heads (head_size 64),
  qkv = x @ w_attn + b_attn; causal softmax attention; y @ w_proj + b_proj.

Sharding: 8 cores = 2 (batch) x 4 (head groups of 4 heads).  Core c handles
batch b = c // 4 and heads [4*hg, 4*hg + 4) with hg = c % 4.  The projection
is row-parallel (each core contracts its 256 y-columns against its w_proj row
slice), so each core emits a partial [1024, 2048] outT; the host sums the 4
partials per batch, transposes, and adds b_proj.

Layout trick: the host feeds x[b].T (bf16) so every on-chip matmul consumes
natural layouts (contraction on partitions) and no PE transposes are needed:
  qkT[c', t]  = wqk.T @ xT           (lhsT = wqk [C, 512],  rhs = xT)
  v'[t, d']   = xT.T  @ wv           (lhsT = xT, rhs = wv; 65 cols per head,
                                      column 64 preset to 1.0)
  sT[j, i]    = kT_h.T @ qT_h        (K = 64, two heads row-packed into the
                                      PE array; one 2-bank PSUM tile per pair)
  attT        = exp(sT / 8)          (one ScalarE op per pair; causal mask via
                                      gpsimd affine_select on diagonal blocks)
  yT'[d', i]  = v'_h.T @ attT_h      (M = 65: row 64 accumulates the softmax
                                      denominator for free)
  yT          = yT'[0:64] / yT'[64]  (DVE recip -> partition_broadcast -> mult)
  outT[c, t]  = wp.T @ yT            (lhsT = wp [256, 1024], rhs = yT)
All matmuls are bf16 with fp32 PSUM accumulation; causality skips 24 of 64
S^T/AV block-columns; the attention inner loop is software-pipelined so PE
issues S^T(jt+1) while ScalarE computes exp(jt).
"""

import ml_dtypes
import numpy as np

P = 128
B, T, C = 2, 2048, 1024
N_HEAD = 16
HSZ = C // N_HEAD          # 64
HG = 4                     # heads per core
DQK = 2 * HG * HSZ         # 512 (q cols + k cols per core)
DV = HG * HSZ              # 256 (v cols per core)
KSUB = C // P              # 8  k-subtiles for the C contraction
ICH = 512                  # i-chunk (PSUM free dim)
NIC = T // ICH             # 4
NJT = T // P               # 16 j-tiles
SCALE = 1.0 / np.sqrt(HSZ)  # 0.125

_CACHE = {}


def _build(reps=1, loop_reps=1, body="full"):
    """body: 'full' (the real kernel), or diagnostics: 'noio' (input DMAs
    hoisted out of the timing loop, no output DMA), 'empty' (tiny DMA
    round-trip only — measures For_i loop overhead)."""
    import concourse.bacc as bacc
    import concourse.mybir as mybir
    import concourse.tile as tile

    f32 = mybir.dt.float32
    bf16 = mybir.dt.bfloat16
    f32r = mybir.dt.float32r
    AF = mybir.ActivationFunctionType
    ALU = mybir.AluOpType

    nc = bacc.Bacc("TRN2", debug=False, target_bir_lowering=False)

    xT_d = nc.dram_tensor("xT", [C, T], bf16, kind="ExternalInput").ap()
    # m-major host layout: wqk[m, p, ko*128+c] = W[ko*128+p, m*128+c] so each
    # 128-col m-block loads as ONE DMA with 2KB-contiguous segments per
    # partition on both sides
    wqk_d = nc.dram_tensor("wqk", [DQK // P, P, C], bf16,
                           kind="ExternalInput").ap()
    wv_d = nc.dram_tensor("wv", [C, DV], bf16, kind="ExternalInput").ap()
    wp_d = nc.dram_tensor("wp", [DV, C], bf16, kind="ExternalInput").ap()
    bqk_d = nc.dram_tensor("bqk", [DQK], f32, kind="ExternalInput").ap()
    bv_d = nc.dram_tensor("bv", [DV], f32, kind="ExternalInput").ap()
    out_d = nc.dram_tensor("outT", [C, T], bf16, kind="ExternalOutput").ap()

    if body == "empty":
        with tile.TileContext(nc) as tc:
            with tc.tile_pool(name="p", bufs=2) as pool:
                _hints = (mybir.EngineType.PE, mybir.EngineType.DVE,
                          mybir.EngineType.Activation, mybir.EngineType.Pool,
                          mybir.EngineType.SP)
                from contextlib import nullcontext
                loop_ctx = (tc.For_i(0, loop_reps, 1, hint_engines=_hints)
                            if loop_reps > 1 else nullcontext())
                with loop_ctx:
                    t = pool.tile([1, 64], bf16, name="t")
                    nc.sync.dma_start(t[:], xT_d[0:1, 0:64])
                    nc.sync.dma_start(out_d[0:1, 0:64], t[:])
        nc.compile()
        return nc

    if body in ("dma1", "dma2"):
        # DMA bandwidth probes: per-iteration transfer of the full 4MB xT
        # through one hwdge queue (dma1) or split across SP+Activation (dma2)
        with tile.TileContext(nc) as tc:
            with tc.tile_pool(name="p", bufs=1) as pool:
                _hints = (mybir.EngineType.PE, mybir.EngineType.DVE,
                          mybir.EngineType.Activation, mybir.EngineType.Pool,
                          mybir.EngineType.SP)
                from contextlib import nullcontext
                loop_ctx = (tc.For_i(0, loop_reps, 1, hint_engines=_hints)
                            if loop_reps > 1 else nullcontext())
                xt = pool.tile([P, KSUB, T], bf16, name="xt")
                xT_r = xT_d.rearrange("(ko p) t -> p ko t", p=P)
                with loop_ctx:
                    for k in range(0, KSUB, 2):
                        eng = (nc.sync if (body == "dma1" or (k // 2) % 2 == 0)
                               else nc.scalar)
                        eng.dma_start(xt[:, k:k + 2, :], xT_r[:, k:k + 2, :])
                    nc.sync.dma_start(out_d[0:1, 0:64], xt[0:1, 0, 0:64])
        nc.compile()
        return nc

    with tile.TileContext(nc) as tc:
        with (
            tc.tile_pool(name="consts", bufs=1) as consts,
            tc.tile_pool(name="attp", bufs=10) as attp,
            tc.tile_pool(name="recp", bufs=6) as recp,
            tc.tile_pool(name="obp", bufs=2) as obp,
            tc.tile_pool(name="bcp", bufs=4) as bcp,
            tc.tile_pool(name="st_ps", bufs=2, space="PSUM") as st_ps,
            tc.tile_pool(name="yt_ps", bufs=2, space="PSUM") as yt_ps,
            tc.tile_pool(name="pj_ps", bufs=2, space="PSUM") as pj_ps,
        ):
          from contextlib import nullcontext
          _hints = (mybir.EngineType.PE, mybir.EngineType.DVE,
                    mybir.EngineType.Activation, mybir.EngineType.Pool,
                    mybir.EngineType.SP)
          loop_ctx = (tc.For_i(0, loop_reps, 1, hint_engines=_hints)
                      if loop_reps > 1 else nullcontext())

          def load_inputs():
            # ---------------- input DMA (ordered by first use) ----------------
            xt = consts.tile([P, KSUB, T], bf16, name="xt")
            xT_r = xT_d.rearrange("(ko p) t -> p ko t", p=P)
            wqk = consts.tile([P, DQK // P, KSUB, P], bf16, name="wqk")
            # load order = first use: the first attention call gates on
            # m0 (q01) + m2 (k01) + xt cc0 + wv/bqk only (~2.5 MB); m1/m3
            # backfill right after.  Every DMA moves >=1KB-contiguous
            # segments per partition (sub-KB scatter throttles the queue).
            bqk = consts.tile([P, DQK // P], f32, name="bqk")
            wv = consts.tile([P, KSUB, DV], bf16, name="wv")
            bv_row = consts.tile([1, DV], f32, name="bv_row")

            def wqk_m(m, half=None):
                ksl = (slice(None) if half is None
                       else slice(half * (KSUB // 2), (half + 1) * (KSUB // 2)))
                wr = wqk_d[m].rearrange("p (ko c) -> p ko c", c=P)
                nc.sync.dma_start(wqk[:, m, ksl], wr[:, ksl])
            nc.sync.dma_start(xt[:, 0:2, 0:ICH], xT_r[:, 0:2, 0:ICH])
            wqk_m(0, 0)
            nc.sync.dma_start(bqk[:], bqk_d.rearrange("(m p) -> p m", p=P))
            nc.sync.dma_start(xt[:, 2:4, 0:ICH], xT_r[:, 2:4, 0:ICH])
            wqk_m(0, 1)
            nc.sync.dma_start(wv[:], wv_d.rearrange("(ko p) n -> p ko n", p=P))
            nc.sync.dma_start(xt[:, 4:6, 0:ICH], xT_r[:, 4:6, 0:ICH])
            wqk_m(2, 0)
            nc.sync.dma_start(xt[:, 6:8, 0:ICH], xT_r[:, 6:8, 0:ICH])
            wqk_m(2, 1)
            wqk_m(1)
            wqk_m(3)
            nc.sync.dma_start(bv_row[:], bv_d[None, :])
            for cc in range(1, NIC):
                nc.sync.dma_start(
                    xt[:, :, cc * ICH:(cc + 1) * ICH],
                    xT_r[:, :, cc * ICH:(cc + 1) * ICH],
                )
            wp = consts.tile([P, DV // P, C], bf16, name="wp")
            nc.sync.dma_start(wp[:], wp_d.rearrange("(ko p) m -> p ko m", p=P))
            bv_bc = consts.tile([P, DV], f32, name="bv_bc")
            nc.gpsimd.partition_broadcast(bv_bc[:], bv_row[:])
            return xt, wqk, bqk, wv, wp, bv_bc

          if body == "noio":
            xt, wqk, bqk, wv, wp, bv_bc = load_inputs()

          # ---- loop-invariant setup (outside For_i: built exactly once) ----
          # preload the Exp activation table so the first real exp doesn't
          # pay the 1.3us table load
          warm = consts.tile([1, 2], f32, name="warm")
          nc.gpsimd.memset(warm[:], 0.0)
          nc.scalar.activation(warm[0:1, 0:1], warm[0:1, 1:2],
                               AF.Exp, scale=SCALE)
          # persistent activations
          qk = consts.tile([P, 4, T], bf16, name="qk")     # m: q01 q23 k01 k23
          v = consts.tile([P, NJT, 4 * (HSZ + 1)], bf16, name="v")
          # only the c=64 column of each head block must be 1.0 (softmax
          # denominator rides the extra matmul row); the rest is written by
          # the bias add
          nc.gpsimd.memset(
              v[:].rearrange("p j (h c) -> p j h c", c=HSZ + 1)[:, :, :, HSZ:],
              1.0)
          yt = consts.tile([P, 2, T], bf16, name="yt")

          with loop_ctx:
           for _rep in range(reps):
            if body in ("full", "serial"):
                xt, wqk, bqk, wv, wp, bv_bc = load_inputs()
            if body == "serial":
                # cross-iteration serializer (bench diagnostic only): gate the
                # first compute of this iteration on the previous iteration's
                # final output DMA, so the For_i slope measures serial latency
                # rather than pipelined throughput.
                dummy = recp.tile([1, 64], bf16, tag="rec", name="dummy")
                nc.sync.dma_start(dummy[:], out_d[None, 0, T - 64:T])
                zb = recp.tile([1, 1], bf16, tag="rec", name="zb")
                nc.vector.tensor_scalar_mul(zb[:], dummy[0:1, 0:1], 0.0)
                nc.vector.tensor_tensor(wqk[0:1, 0, 0, 0:1],
                                        wqk[0:1, 0, 0, 0:1], zb[:], ALU.add)

            # ---- phase emitters (order below controls PE stream / overlap) ----
            def emit_qkT(m, cc, evict_act=False):
                # shares the pj pool (1-bank tiles) so fillers never steal the
                # attention pipeline's st_ps buffers
                ps = pj_ps.tile([P, ICH], f32, tag="pj", name="qk_ps")
                for k in range(KSUB):
                    nc.tensor.matmul(
                        ps[:],
                        lhsT=wqk[:, m, k, :],
                        rhs=xt[:, k, cc * ICH:(cc + 1) * ICH],
                        start=(k == 0),
                        stop=(k == KSUB - 1),
                    )
                if evict_act:
                    # first q01/k01 evicts ride the idle Activation engine
                    # (per-partition bias == channel bias), keeping the
                    # attention-start chain off the busier DVE queue
                    nc.scalar.activation(
                        qk[:, m, cc * ICH:(cc + 1) * ICH], ps[:],
                        AF.Identity, bias=bqk[:, m:m + 1], scale=1.0)
                else:
                    nc.vector.tensor_tensor(
                        qk[:, m, cc * ICH:(cc + 1) * ICH], ps[:],
                        bqk[:, m:m + 1].to_broadcast([P, ICH]), ALU.add,
                    )

            def emit_v(t):
                ps = pj_ps.tile([P, ICH], f32, tag="pj", name="v_ps")
                for k in range(KSUB):
                    nc.tensor.matmul(
                        ps[0:P, 0:DV],
                        lhsT=xt[:, k, t * P:(t + 1) * P],
                        rhs=wv[:, k, :],
                        start=(k == 0),
                        stop=(k == KSUB - 1),
                    )
                nc.vector.tensor_tensor(
                    v[:, t, :].rearrange("p (h c) -> p h c", c=HSZ + 1)[:, :, 0:HSZ],
                    ps[0:P, 0:DV].rearrange("p (h c) -> p h c", c=HSZ),
                    bv_bc[:].rearrange("p (h c) -> p h c", c=HSZ),
                    ALU.add,
                )

            def emit_attn(ic, hps, extra=(), direct_norm=False):
                """Attention for i-chunk ic over head-pairs in hps.  `extra`
                is a list of thunks (projection units of the previous i-chunk)
                emitted one per j-tile step so the PE has filler work while
                ScalarE computes the exp."""
                isl = slice(ic * ICH, (ic + 1) * ICH)
                njt = 4 * ic + 4          # causal: j-tiles 0 .. 4*ic+3
                HB = HSZ + 1
                extra = list(extra)
                ytp = {hp: (yt_ps.tile([P, ICH], f32, tag="yt", name="ytpA"),
                            yt_ps.tile([P, ICH], f32, tag="yt", name="ytpB"))
                       for hp in hps}

                def emit_st(hp, jt):
                    jsl = slice(jt * P, (jt + 1) * P)
                    # diagonal blocks: columns f < 128r are masked for every
                    # partition, so compute only the valid suffix [n0:ICH)
                    r = jt - 4 * ic if jt >= 4 * ic else None
                    n0 = 0 if r is None else P * r
                    ssl = slice(ic * ICH + n0, (ic + 1) * ICH)
                    st2 = st_ps.tile([P, 2, ICH], f32, tag="st", name="st2")
                    for h in range(2):
                        hsl = slice(64 * h, 64 * h + 64)
                        nc.tensor.matmul(
                            st2[:, h, n0:],
                            lhsT=qk[hsl, 2 + hp, jsl],
                            rhs=qk[hsl, hp, ssl],
                        )
                    a2 = attp.tile([P, 2, ICH], bf16, tag="att", name="a2")
                    nc.scalar.activation(a2[:, :, n0:], st2[:, :, n0:],
                                         AF.Exp, scale=SCALE)
                    if r is not None:
                        # only the 128-wide band [n0, n0+128) is partial; the
                        # rest of the suffix is fully valid.  within the band
                        # keep f >= p.  Runs on Pool, off the PE pipeline; the
                        # depth-2 AV delay hides its latency.
                        mb = slice(n0, min(n0 + P, ICH))
                        nc.gpsimd.affine_select(
                            out=a2[:, :, mb], in_=a2[:, :, mb],
                            compare_op=ALU.is_ge, fill=0.0,
                            base=0, channel_multiplier=-1,
                            pattern=[[0, 2], [1, mb.stop - mb.start]])
                    return a2, n0

                def emit_av(hp, jt, a2, n0):
                    first, last = jt == 0, jt == njt - 1
                    ytpA, ytpB = ytp[hp]
                    nc.tensor.matmul(
                        ytpA[0:HB, n0:],
                        lhsT=v[:, jt, (2 * hp) * HB:(2 * hp + 1) * HB],
                        rhs=a2[:, 0, n0:],
                        start=first, stop=last,
                    )
                    nc.tensor.matmul(
                        ytpB[0:HB, n0:],
                        lhsT=v[:, jt, (2 * hp + 1) * HB:(2 * hp + 2) * HB],
                        rhs=a2[:, 1, n0:],
                        start=first, stop=last,
                    )

                # software pipeline depth 2: AV(jt) issues two steps after its
                # S^T, so the Act->PE semaphore latency is absorbed by the
                # queue instead of stalling the PE
                pend = []
                for jt in range(njt):
                    cur = [(hp,) + tuple(emit_st(hp, jt)) for hp in hps]
                    if len(pend) >= 2:
                        pjt, pcur = pend.pop(0)
                        for hp, a2, n0 in pcur:
                            emit_av(hp, pjt, a2, n0)
                    if extra:
                        extra.pop(0)()
                    pend.append((jt, cur))
                for pjt, pcur in pend:
                    for hp, a2, n0 in pcur:
                        emit_av(hp, pjt, a2, n0)
                for th in extra:
                    th()

                for hp in hps:
                    ytpA, ytpB = ytp[hp]
                    if direct_norm:
                        # final attn call: no successor competes for these
                        # psum banks, so normalize straight out of PSUM —
                        # shortens the chain into the final projection
                        ycA, ycB = ytpA, ytpB
                    else:
                        # fast-evict to SBUF: frees the psum accumulators;
                        # keeps ScalarE free for the exp stream
                        ycA = recp.tile([P, ICH], f32, tag="rec", name="ycA")
                        ycB = recp.tile([P, ICH], f32, tag="rec", name="ycB")
                        nc.vector.tensor_copy(ycA[0:HB, :], ytpA[0:HB, :])
                        nc.vector.tensor_copy(ycB[0:HB, :], ytpB[0:HB, :])
                    # shifted DVE write puts the recip at partition 0 of a
                    # fresh tile, where partition_broadcast needs its source
                    recA = bcp.tile([P, ICH], f32, tag="rec2", name="recA")
                    recB = bcp.tile([P, ICH], f32, tag="rec2", name="recB")
                    nc.vector.reciprocal(recA[0:1, :], ycA[64:65, :])
                    nc.vector.reciprocal(recB[0:1, :], ycB[64:65, :])
                    # only partitions 0:64 are read by the mults below
                    rbA = bcp.tile([P, ICH], f32, tag="rb", name="rbA")
                    rbB = bcp.tile([P, ICH], f32, tag="rb", name="rbB")
                    nc.gpsimd.partition_broadcast(rbA[0:64, :], recA[0:1, :])
                    nc.gpsimd.partition_broadcast(rbB[0:64, :], recB[0:1, :])
                    nc.vector.tensor_tensor(yt[0:64, hp, isl], ycA[0:64, :],
                                            rbA[0:64, :], ALU.mult)
                    nc.vector.tensor_tensor(yt[64:128, hp, isl], ycB[0:64, :],
                                            rbB[0:64, :], ALU.mult)

            def proj_units(ic, stream_dma=False, split_k_first=0):
                """split_k_first: that many leading m-units are emitted as
                (k2=0 half now, k2=1 half at the next unit's slot) so their
                first halves run while the hp1 normalize chain drains —
                their psum tiles stay live across the gap (needs <= pj bufs).
                """
                isl = slice(ic * ICH, (ic + 1) * ICH)
                ob = obp.tile([P, C // P, ICH], bf16, tag="ob", name="ob")
                out_r = out_d.rearrange("(m p) t -> p m t", p=P)
                held = {}

                def mm(pj, m, k2):
                    nc.tensor.matmul(
                        pj[:],
                        lhsT=wp[:, k2, m * P:(m + 1) * P],
                        rhs=yt[:, k2, isl],
                        start=(k2 == 0),
                        stop=(k2 == DV // P - 1),
                    )

                def finish(m, pj):
                    nc.vector.tensor_copy(ob[:, m, :], pj[:])
                    if body in ("full", "serial"):
                        if stream_dma and (m + 1) % stream_dma == 0:
                            # late chunks: stream out in m-groups so the
                            # final DMA is small, not 2 MB of tail
                            lo = m + 1 - stream_dma
                            nc.sync.dma_start(
                                out_r[:, lo:m + 1, isl], ob[:, lo:m + 1, :])
                        elif not stream_dma and m == C // P - 1:
                            # one batched DMA per i-chunk: 8 m-tiles
                            nc.sync.dma_start(out_r[:, :, isl], ob[:])

                def unit(m):
                    def th():
                        if m < split_k_first:
                            # first half only (k2=0, hp0 — no normalize dep)
                            pj = pj_ps.tile([P, ICH], f32, tag="pj", name="pj")
                            mm(pj, m, 0)
                            held[m] = pj
                            return
                        if m - split_k_first in held:
                            pjh = held.pop(m - split_k_first)
                            mm(pjh, m - split_k_first, 1)
                            finish(m - split_k_first, pjh)
                        pj = pj_ps.tile([P, ICH], f32, tag="pj", name="pj")
                        for k2 in range(DV // P):
                            mm(pj, m, k2)
                        finish(m, pj)
                        if m == C // P - 1:
                            for mh in sorted(held):
                                pjh = held.pop(mh)
                                mm(pjh, mh, 1)
                                finish(mh, pjh)
                    return th
                return [unit(m) for m in range(C // P)]

            # ---- emission order: start attention (ScalarE exp) early, and
            # spread every PE-only unit (qkT cc>=1, v jt>=4, projection) as
            # per-jt filler inside the attention loops so the PE never idles
            # while ScalarE drains the exp backlog ----
            def qkT_unit(m, cc):
                return lambda: emit_qkT(m, cc)

            def v_unit(t):
                return lambda: emit_v(t)

            nop = lambda: None  # noqa: E731  boundary slot: let normalize land

            # projections are delayed one full i-chunk (proj(ic) runs as
            # filler during attn(ic+2)) so they never sit behind a freshly
            # issued normalize chain
            emit_qkT(0, 0)
            emit_qkT(2, 0)
            # v(0) rides as the first filler: the attention gate then needs
            # only m0 + m2 + xt cc0 (~2 MB); wv streams in behind
            emit_attn(0, [0], extra=[v_unit(0), v_unit(1), v_unit(2),
                                     v_unit(3), qkT_unit(1, 0)])
            emit_qkT(3, 0)
            emit_attn(0, [1], extra=[qkT_unit(0, 1), qkT_unit(2, 1),
                                     v_unit(4), v_unit(5)])
            emit_attn(1, [0], extra=[qkT_unit(1, 1), qkT_unit(3, 1),
                                     v_unit(6), v_unit(7),
                                     qkT_unit(0, 2), qkT_unit(2, 2)])
            emit_attn(1, [1], extra=[qkT_unit(1, 2), qkT_unit(3, 2),
                                     v_unit(8), v_unit(9)])
            pu0 = proj_units(0)
            emit_attn(2, [0], extra=[qkT_unit(0, 3), qkT_unit(2, 3),
                                     v_unit(10), v_unit(11)] + pu0[:4])
            emit_attn(2, [1], extra=[qkT_unit(1, 3), qkT_unit(3, 3),
                                     v_unit(12), v_unit(13)] + pu0[4:])
            pu1 = proj_units(1)
            emit_attn(3, [0], extra=[nop, v_unit(14), v_unit(15)] + pu1[:6])
            pu2 = proj_units(2, stream_dma=2)
            emit_attn(3, [1], extra=pu1[6:] + pu2[:4], direct_norm=True)
            # pu2 leftovers keep the PE fed while the final normalize chain
            # (DVE recip -> Pool broadcast -> DVE mult) drains
            for th in pu2[4:]:
                th()
            for th in proj_units(NIC - 1, stream_dma=1):
                th()

    nc.compile()
    return nc


def _get_nc(reps=1, loop_reps=1, body="full"):
    key = ("nc", reps, loop_reps, body)
    if key not in _CACHE:
        _CACHE[key] = _build(reps, loop_reps, body)
    return _CACHE[key]


def _shard_inputs(x, w_attn, b_attn, w_proj, b_proj):
    x = np.asarray(x, dtype=np.float32)
    w_attn = np.asarray(w_attn, dtype=np.float32)
    b_attn = np.asarray(b_attn, dtype=np.float32)
    w_proj = np.asarray(w_proj, dtype=np.float32)
    b_proj = np.asarray(b_proj, dtype=np.float32)

    xTs = [np.ascontiguousarray(x[b].T.astype(ml_dtypes.bfloat16)) for b in range(B)]
    in_maps = []
    for core in range(8):
        b, hg = divmod(core, 4)
        q = slice(hg * DV, (hg + 1) * DV)
        k = slice(C + hg * DV, C + (hg + 1) * DV)
        vs = slice(2 * C + hg * DV, 2 * C + (hg + 1) * DV)
        wqk_cm = np.concatenate([w_attn[:, q], w_attn[:, k]], axis=1)  # [C,512]
        # m-major device layout: [m, p, ko*128+c] = wqk_cm[ko*128+p, m*128+c]
        wqk_m = (wqk_cm.reshape(KSUB, P, 4, P).transpose(2, 1, 0, 3)
                 .reshape(4, P, C))
        in_maps.append({
            "xT": xTs[b],
            "wqk": np.ascontiguousarray(wqk_m.astype(ml_dtypes.bfloat16)),
            "wv": np.ascontiguousarray(w_attn[:, vs].astype(ml_dtypes.bfloat16)),
            "wp": np.ascontiguousarray(
                w_proj[hg * DV:(hg + 1) * DV, :].astype(ml_dtypes.bfloat16)),
            "bqk": np.ascontiguousarray(
                np.concatenate([b_attn[q], b_attn[k]])),
            "bv": np.ascontiguousarray(b_attn[vs]),
        })
    return in_maps, b_proj


def _unshard(results, b_proj):
    out = np.zeros((B, T, C), dtype=np.float32)
    for core in range(8):
        b = core // 4
        out[b] += results[core]["outT"].T.astype(np.float32)
    out += b_proj[None, None, :]
    return out


def _run(inputs, **kwargs):
    from concourse.bass_utils import run_bass_kernel_spmd

    nc = _get_nc()
    in_maps, b_proj = _shard_inputs(**inputs)
    res = run_bass_kernel_spmd(nc, in_maps, core_ids=list(range(8)), **kwargs)
    return res, _unshard(res.results, b_proj)


def kernel(x, w_attn, b_attn, w_proj, b_proj):
    _, out = _run(dict(x=x, w_attn=w_attn, b_attn=b_attn,
                       w_proj=w_proj, b_proj=b_proj))
    return out

